# revision 14
# baseline (speedup 1.0000x reference)
"""Causal self-attention (B=2, T=4096, C=768, H=12) on 8 trn2 NeuronCores.

Sharding: data-parallel on batch (cores 0-3 -> batch 0, cores 4-7 -> batch 1),
tensor-parallel on heads (3 heads per core).  Each core computes qkv for its
3 heads, causal flash-style attention, and a partial output projection
(its heads' rows of w_proj); the host sums the 4 partials per batch.

v8: all matmul operands in bf16 (inputs pre-converted on host), which
removes the fp32->fp32r rounding copies, halves x DMA, and doubles DVE
rate on the causal-mask multiplies.  Attention S tiles are computed with
per-tile causal column windows (diagonal tiles only compute queries that
can see the tile's keys), packed two-to-a-batch into PSUM so each exp
instruction covers ~1K columns.  The softmax normalization runs in the
keys-major layout: reciprocal of the sum row, PE broadcast of it across
64 partitions, one tensor-mul into Y^T -- no transposes in the tail.
"""

import sys

if '/opt/trn_rl_repo' not in sys.path:
    sys.path.insert(0, '/opt/trn_rl_repo')

import numpy as np

import concourse.bacc as bacc
import concourse.mybir as mybir
import concourse.tile as tile

dt = mybir.dt
F32 = dt.float32
F32R = dt.float32r
BF16 = dt.bfloat16

N_EMBD = 768
N_HEADS = 12
HEAD_DIM = 64
B = 2
T_FULL = 4096
N_CORES = 8
HEADS_PER_CORE = N_HEADS // (N_CORES // B)  # 3

TOK_CHUNK = 1024  # qkv phase DMA chunk (2KB bf16 rows)
SUB = 512         # qkv matmul moving width
QSB = 512         # attention query superblock
KT = 128          # key tile (contraction for P@V)
CCHUNKS = N_EMBD // 128  # 6 contraction chunks


def build_nc(T=T_FULL, repeat=1, phases=('B', 'B2', 'C', 'D')):
    """Build the per-core Bass program.  Same program runs SPMD on all 8
    cores; per-core data (x^T of its batch in bf16, its heads' weight
    slices in bf16) comes via the input map."""
    nc = bacc.Bacc(None, target_bir_lowering=False, debug=False)

    n_kt = T // KT
    n_qsb = T // QSB
    n_tok = T // 128

    XT = nc.dram_tensor("xt", [N_EMBD, T], BF16, kind="ExternalInput")
    WQ01 = nc.dram_tensor("wq01", [N_EMBD, 128], BF16, kind="ExternalInput")
    WK01 = nc.dram_tensor("wk01", [N_EMBD, 128], BF16, kind="ExternalInput")
    WV = nc.dram_tensor("wv", [N_EMBD, 192], BF16, kind="ExternalInput")
    WQ2 = nc.dram_tensor("wq2", [N_EMBD, 64], BF16, kind="ExternalInput")
    WK2 = nc.dram_tensor("wk2", [N_EMBD, 64], BF16, kind="ExternalInput")
    WP1 = nc.dram_tensor("wp1", [128, N_EMBD], BF16, kind="ExternalInput")
    WP2 = nc.dram_tensor("wp2", [64, N_EMBD], BF16, kind="ExternalInput")
    Y = nc.dram_tensor("y", [T, N_EMBD], F32, kind="ExternalOutput")

    xt_ap = XT.ap().rearrange("(c p) t -> p c t", p=128)

    with tile.TileContext(nc) as tc:
        with (
            tc.tile_pool(name="const", bufs=1) as const_pool,
            tc.tile_pool(name="wpool", bufs=1) as wpool,
            tc.tile_pool(name="qkvt", bufs=1) as qkvt,
            tc.tile_pool(name="vsb", bufs=1) as vsb_pool,
            tc.tile_pool(name="ynt", bufs=1) as ynt_pool,
            tc.tile_pool(name="xs", bufs=3) as xs_pool,
            tc.tile_pool(name="ptp", bufs=4) as pt_pool,
            tc.tile_pool(name="rp", bufs=4) as r_pool,
            tc.tile_pool(name="yout", bufs=4) as yout_pool,
            tc.tile_pool(name="pbig", bufs=2, space="PSUM") as pbig,
            tc.tile_pool(name="pf", bufs=2, space="PSUM") as pf_pool,
            tc.tile_pool(name="py", bufs=2, space="PSUM") as py_pool,
        ):
            # ---- constants ----
            # causal triangle for diagonal 128x128 blocks: T[p, c] = 1 iff
            # c >= p (query col c sees key row p).
            tri_f = const_pool.tile([128, 128], F32, name="tri_f")
            nc.gpsimd.memset(tri_f, 1.0)
            nc.gpsimd.affine_select(
                out=tri_f, in_=tri_f,
                compare_op=mybir.AluOpType.is_ge,
                fill=0.0, base=0, channel_multiplier=-1,
                pattern=[[1, 128]],
            )
            tri = const_pool.tile([128, 128], BF16, name="tri")
            nc.vector.tensor_copy(out=tri, in_=tri_f)
            ones_rf = const_pool.tile([1, 64], F32, name="ones_rf")
            nc.vector.memset(ones_rf, 1.0)
            ones_r = const_pool.tile([1, 64], F32R, name="ones_r")
            nc.vector.tensor_copy(out=ones_r, in_=ones_rf)

            # ---- weights (bf16, direct load) ----
            def load_w(src_ap, shape, tag):
                t = wpool.tile(shape, BF16, tag=tag)
                nc.sync.dma_start(out=t, in_=src_ap)
                return t

            wq01r = load_w(WQ01.ap().rearrange("(c p) m -> p c m", p=128), [128, CCHUNKS, 128], "wq01r")
            wk01r = load_w(WK01.ap().rearrange("(c p) m -> p c m", p=128), [128, CCHUNKS, 128], "wk01r")
            wvr = load_w(WV.ap().rearrange("(c p) m -> p c m", p=128), [128, CCHUNKS, 192], "wvr")
            wq2r = load_w(WQ2.ap().rearrange("(c p) m -> p c m", p=128), [128, CCHUNKS, 64], "wq2r")
            wk2r = load_w(WK2.ap().rearrange("(c p) m -> p c m", p=128), [128, CCHUNKS, 64], "wk2r")
            wp1r = load_w(WP1.ap(), [128, N_EMBD], "wp1r")
            wp2r = load_w(WP2.ap(), [64, N_EMBD], "wp2r")

            # ---- persistent activations (bf16) ----
            QT01 = qkvt.tile([128, T], BF16, tag="qt01")
            KT01 = qkvt.tile([128, T], BF16, tag="kt01")
            Q2 = qkvt.tile([64, T], BF16, tag="q2")
            KT2 = qkvt.tile([64, T], BF16, tag="kt2")
            Vsb = vsb_pool.tile([128, n_kt, HEADS_PER_CORE, 65], BF16)
            YnT01 = ynt_pool.tile([128, T], BF16, tag="ynt01")
            YnT2 = ynt_pool.tile([64, T], BF16, tag="ynt2")

            ones_f = const_pool.tile([128, n_kt * HEADS_PER_CORE], F32)
            nc.vector.memset(ones_f, 1.0)
            nc.vector.tensor_copy(
                out=Vsb[:, :, :, 64:65].rearrange("p a b c -> p (a b c)"),
                in_=ones_f)

            scale = float(HEAD_DIM) ** -0.5

            # K jobs run early (every later query block needs them);
            # Q jobs are deferred to just before their query block.
            # V is computed directly in keys-major layout (v_unit below).
            kv_jobs = [(wk01r, KT01, 128), (wk2r, KT2, 64)]
            q_jobs = [(wq01r, QT01, 128), (wq2r, Q2, 64)]
            head_qk = [
                (QT01[0:64, :], KT01[0:64, :]),
                (QT01[64:128, :], KT01[64:128, :]),
                (Q2[0:64, :], KT2[0:64, :]),
            ]

            # ---- phase-B/B2/D work units (interleaved as PE filler) ----
            def b_unit(xs, ch, s, job, jobs):
                wt, out_sb, m = jobs[job]
                t0 = ch * TOK_CHUNK + s * SUB
                ps = pf_pool.tile([128, SUB], F32, tag="pf")
                for c in range(CCHUNKS):
                    nc.tensor.matmul(
                        ps[0:m, :], wt[:, c, 0:m],
                        xs[:, c, s * SUB:(s + 1) * SUB],
                        start=(c == 0), stop=(c == CCHUNKS - 1))
                nc.vector.tensor_copy(out=out_sb[0:m, t0:t0 + SUB], in_=ps[0:m, :])

            def v_unit(xs, ch, kl):
                # V for one 128-key tile, all 3 heads, directly keys-major:
                # stationary = x tokens, moving = V weight columns.
                kt = ch * (TOK_CHUNK // KT) + kl
                pv = pf_pool.tile([128, 3, 64], F32, tag="pf")
                pv_flat = pv.rearrange("p h c -> p (h c)")
                for c in range(CCHUNKS):
                    nc.tensor.matmul(
                        pv_flat, xs[:, c, kl * KT:(kl + 1) * KT],
                        wvr[:, c, :],
                        start=(c == 0), stop=(c == CCHUNKS - 1))
                nc.vector.tensor_copy(out=Vsb[:, kt, :, 0:64], in_=pv)

            def d_unit(tt):
                tsl = slice(tt * 128, (tt + 1) * 128)
                yo = yout_pool.tile([128, N_EMBD], F32)
                for c0, ncols in ((0, 512), (512, 256)):
                    pp = pf_pool.tile([128, 512], F32, tag="pf")
                    nc.tensor.matmul(pp[:, 0:ncols], YnT01[:, tsl],
                                     wp1r[:, c0:c0 + ncols], start=True, stop=False)
                    nc.tensor.matmul(pp[:, 0:ncols], YnT2[0:64, tsl],
                                     wp2r[0:64, c0:c0 + ncols], start=False, stop=True)
                    nc.vector.tensor_copy(out=yo[:, c0:c0 + ncols], in_=pp[:, 0:ncols])
                nc.sync.dma_start(out=Y.ap()[tsl, :], in_=yo)

            # ---- attention k-loop: software-pipelined emission ----
            def kloop(h, qs, yps):
                """One yield per batch of <=2 key tiles.  Each step emits
                S+exp of batch k, then mask+PV of batch k-1, so PE never
                sits behind ACT in program order.  Diagonal tiles are
                column-windowed and triangle-masked after exp."""
                qt_h, kt_h = head_qk[h]
                nkt = 4 * (qs + 1)
                qbase = qs * QSB
                batches = []
                full = list(range(nkt - 4))
                for i in range(0, len(full), 2):
                    batches.append([(kt, 0) for kt in full[i:i + 2]])
                batches.append([(nkt - 4, 0), (nkt - 3, 128)])
                batches.append([(nkt - 2, 256), (nkt - 1, 384)])

                def mask_pv(batch, pt):
                    off = 0
                    for (kt, d) in batch:
                        w = QSB - d
                        if kt >= nkt - 4:
                            nc.vector.tensor_mul(
                                pt[:, off:off + 128], pt[:, off:off + 128], tri)
                        off += w
                    off = 0
                    for (kt, d) in batch:
                        w = QSB - d
                        nc.tensor.matmul(
                            yps[:, d:QSB], Vsb[:, kt, h, :], pt[:, off:off + w],
                            start=(kt == 0), stop=(kt == nkt - 1))
                        off += w

                prev = None
                for batch in batches:
                    widths = [QSB - d for (_, d) in batch]
                    W = sum(widths)
                    sb = pbig.tile([128, 1024], F32, tag="big")
                    off = 0
                    for (kt, d), w in zip(batch, widths):
                        nc.tensor.matmul(
                            sb[:, off:off + w],
                            kt_h[:, kt * KT:(kt + 1) * KT],
                            qt_h[:, qbase + d:qbase + QSB],
                            start=True, stop=True)
                        off += w
                    pt = pt_pool.tile([128, 1024], BF16)
                    nc.scalar.activation(
                        out=pt[:, 0:W], in_=sb[:, 0:W],
                        func=mybir.ActivationFunctionType.Exp, scale=scale)
                    if prev is not None:
                        mask_pv(*prev)
                    prev = (batch, pt)
                    yield
                mask_pv(*prev)

            def finish(h, qs, yps):
                """Normalize in keys-major layout and store Y^T (bf16)."""
                qsl = slice(qs * QSB, (qs + 1) * QSB)
                rrow = r_pool.tile([1, QSB], F32R)
                with nc.allow_low_precision(reason="fp32r storage is fp32 bits"):
                    nc.vector.reciprocal(rrow, yps[64:65, :])
                rbp = pf_pool.tile([64, QSB], F32, tag="pf", name="rbp")
                nc.tensor.matmul(rbp, ones_r, rrow, start=True, stop=True)
                rb = r_pool.tile([64, QSB], F32, tag="rb", name="rb")
                nc.vector.tensor_copy(out=rb, in_=rbp)
                if h == 0:
                    dst = YnT01[0:64, qsl]
                elif h == 1:
                    dst = YnT01[64:128, qsl]
                else:
                    dst = YnT2[0:64, qsl]
                nc.vector.tensor_mul(dst, yps[0:64, :], rb)

            n_chunks = T // TOK_CHUNK

            for _ in range(repeat):
                from collections import deque
                bq, b2q, qq, dq = deque(), deque(), deque(), deque()
                chunks_queued = set()

                def queue_chunk(ch):
                    if ch in chunks_queued or ch >= n_chunks or 'B' not in phases:
                        return
                    chunks_queued.add(ch)
                    xs = xs_pool.tile([128, CCHUNKS, TOK_CHUNK], BF16)
                    nc.sync.dma_start(
                        out=xs,
                        in_=xt_ap[:, :, ch * TOK_CHUNK:(ch + 1) * TOK_CHUNK])
                    for s in range(TOK_CHUNK // SUB):
                        for job in range(len(kv_jobs)):
                            bq.append((ch, lambda xs=xs, ch=ch, s=s, job=job:
                                       b_unit(xs, ch, s, job, kv_jobs)))
                        sub_qs = ch * (TOK_CHUNK // SUB) + s
                        for job in range(len(q_jobs)):
                            qq.append((sub_qs, lambda xs=xs, ch=ch, s=s, job=job:
                                       b_unit(xs, ch, s, job, q_jobs)))
                    if 'B2' in phases:
                        for kl in range(TOK_CHUNK // KT):
                            kt = ch * (TOK_CHUNK // KT) + kl
                            b2q.append((kt, lambda xs=xs, ch=ch, kl=kl:
                                        v_unit(xs, ch, kl)))

                def force_prereqs(qs):
                    """Emit every queued B unit of chunks <= qs//2 and every
                    B2 unit with kt < 4*(qs+1)."""
                    need_ch = min(qs // 2, n_chunks - 1)
                    while bq and bq[0][0] <= need_ch:
                        bq.popleft()[1]()
                    while b2q and b2q[0][0] < 4 * (qs + 1):
                        b2q.popleft()[1]()
                    while qq and qq[0][0] <= qs:
                        qq.popleft()[1]()

                state = {'cur_qs': 0}

                def drain_fillers(n, final=False):
                    for _ in range(n):
                        if bq:
                            bq.popleft()[1]()
                        elif b2q:
                            b2q.popleft()[1]()
                        elif dq:
                            dq.popleft()()
                        elif final and qq:
                            qq.popleft()[1]()
                        else:
                            return

                def queue_d(qs):
                    if 'D' not in phases:
                        return
                    for tt in range(4 * qs, 4 * qs + 4):
                        dq.append(lambda tt=tt: d_unit(tt))

                # prologue: first token chunk (K, V, Q for qs 0-1)
                queue_chunk(0)
                force_prereqs(1)
                queue_chunk(1)

                if 'C' in phases:
                    loop_list = [(h, qs) for qs in range(n_qsb)
                                 for h in range(HEADS_PER_CORE)]
                else:
                    loop_list = []
                    while bq or b2q or qq or dq:
                        drain_fillers(8, final=True)
                active = []
                next_i = 0
                qs_done = {}
                while active or next_i < len(loop_list):
                    while len(active) < 2 and next_i < len(loop_list):
                        h, qs = loop_list[next_i]
                        state['cur_qs'] = qs
                        force_prereqs(qs)
                        queue_chunk(qs // 2 + 1)
                        yps = py_pool.tile([65, QSB], F32, tag="y",
                                           name=f"yps{h}_{qs}")
                        active.append([kloop(h, qs, yps), h, qs, yps])
                        next_i += 1
                    for entry in list(active):
                        gen, h, qs, yps = entry
                        try:
                            next(gen)
                        except StopIteration:
                            finish(h, qs, yps)
                            qs_done[qs] = qs_done.get(qs, 0) + 1
                            if qs_done[qs] == HEADS_PER_CORE:
                                queue_d(qs)
                            active.remove(entry)
                    drain_fillers(2)
                # epilogue: leftover projection units
                while bq or b2q or qq or dq:
                    drain_fillers(8, final=True)

    nc.compile()
    return nc


def make_in_maps(x, w_qkv, w_proj, T=T_FULL):
    """Per-core input dicts from full inputs (numpy), converted to bf16."""
    import ml_dtypes
    bf16 = ml_dtypes.bfloat16
    x = np.asarray(x, dtype=np.float32)
    w_qkv = np.asarray(w_qkv, dtype=np.float32).astype(bf16)
    w_proj = np.asarray(w_proj, dtype=np.float32).astype(bf16)
    cores_per_batch = N_CORES // B
    xt_b = [np.ascontiguousarray(x[b].T.astype(bf16)) for b in range(B)]
    in_maps = []
    for core in range(N_CORES):
        b = core // cores_per_batch
        h0 = (core % cores_per_batch) * HEADS_PER_CORE
        h1, h2 = h0 + 1, h0 + 2
        col = lambda kind, h: w_qkv[:, kind * N_EMBD + h * HEAD_DIM:
                                    kind * N_EMBD + (h + 1) * HEAD_DIM]
        in_maps.append({
            "xt": xt_b[b],
            "wq01": np.ascontiguousarray(np.concatenate([col(0, h0), col(0, h1)], axis=1)),
            "wk01": np.ascontiguousarray(np.concatenate([col(1, h0), col(1, h1)], axis=1)),
            "wv": np.ascontiguousarray(np.concatenate([col(2, h0), col(2, h1), col(2, h2)], axis=1)),
            "wq2": np.ascontiguousarray(col(0, h2)),
            "wk2": np.ascontiguousarray(col(1, h2)),
            "wp1": np.ascontiguousarray(w_proj[h0 * HEAD_DIM:(h1 + 1) * HEAD_DIM, :]),
            "wp2": np.ascontiguousarray(w_proj[h2 * HEAD_DIM:(h2 + 1) * HEAD_DIM, :]),
        })
    return in_maps


def gather_output(results, T=T_FULL):
    cores_per_batch = N_CORES // B
    out = np.empty((B, T, N_EMBD), dtype=np.float32)
    for b in range(B):
        parts = [results[b * cores_per_batch + j]["y"] for j in range(cores_per_batch)]
        out[b] = parts[0] + parts[1] + parts[2] + parts[3]
    return out


_CACHE = {}


def _get_nc(T=T_FULL, repeat=1):
    key = (T, repeat)
    if key not in _CACHE:
        _CACHE[key] = build_nc(T, repeat)
    return _CACHE[key]


def kernel(x, w_qkv, w_proj):
    import time as _time
    from concourse.bass_utils import run_bass_kernel_spmd
    T = x.shape[1]
    nc = _get_nc(T)
    in_maps = make_in_maps(x, w_qkv, w_proj, T)
    last_err = None
    for attempt in range(3):
        try:
            res = run_bass_kernel_spmd(nc, in_maps, list(range(N_CORES)))
            return gather_output(res.results, T)
        except Exception as e:  # transient device wedge: retry after a pause
            last_err = e
            _time.sleep(20 * (attempt + 1))
    raise last_err


# revision 16
# speedup vs baseline: 1.1914x; 1.1914x over previous
"""Causal self-attention (B=2, T=4096, C=768, H=12) on 8 trn2 NeuronCores.

Sharding: data-parallel on batch (cores 0-3 -> batch 0, cores 4-7 -> batch 1),
tensor-parallel on heads (3 heads per core).  Each core computes qkv for its
3 heads, causal flash-style attention, and a partial output projection
(its heads' rows of w_proj); the host sums the 4 partials per batch.

v8: all matmul operands in bf16 (inputs pre-converted on host), which
removes the fp32->fp32r rounding copies, halves x DMA, and doubles DVE
rate on the causal-mask multiplies.  Attention S tiles are computed with
per-tile causal column windows (diagonal tiles only compute queries that
can see the tile's keys), packed two-to-a-batch into PSUM so each exp
instruction covers ~1K columns.  The softmax normalization runs in the
keys-major layout: reciprocal of the sum row, PE broadcast of it across
64 partitions, one tensor-mul into Y^T -- no transposes in the tail.
"""

import sys

if '/opt/trn_rl_repo' not in sys.path:
    sys.path.insert(0, '/opt/trn_rl_repo')

import numpy as np

import concourse.bacc as bacc
import concourse.mybir as mybir
import concourse.tile as tile

dt = mybir.dt
F32 = dt.float32
F32R = dt.float32r
BF16 = dt.bfloat16

N_EMBD = 768
N_HEADS = 12
HEAD_DIM = 64
B = 2
T_FULL = 4096
N_CORES = 8
HEADS_PER_CORE = N_HEADS // (N_CORES // B)  # 3

TOK_CHUNK = 1024  # qkv phase DMA chunk (2KB bf16 rows)
SUB = 512         # qkv matmul moving width
QSB = 512         # attention query superblock
KT = 128          # key tile (contraction for P@V)
CCHUNKS = N_EMBD // 128  # 6 contraction chunks


def build_nc(T=T_FULL, repeat=1, phases=('B', 'B2', 'C', 'D')):
    """Build the per-core Bass program.  Same program runs SPMD on all 8
    cores; per-core data (x^T of its batch in bf16, its heads' weight
    slices in bf16) comes via the input map."""
    nc = bacc.Bacc(None, target_bir_lowering=False, debug=False)

    n_kt = T // KT
    n_qsb = T // QSB
    n_tok = T // 128

    XT = nc.dram_tensor("xt", [N_EMBD, T], BF16, kind="ExternalInput")
    WQ01 = nc.dram_tensor("wq01", [N_EMBD, 128], BF16, kind="ExternalInput")
    WK01 = nc.dram_tensor("wk01", [N_EMBD, 128], BF16, kind="ExternalInput")
    WV = nc.dram_tensor("wv", [N_EMBD, 192], BF16, kind="ExternalInput")
    WQ2 = nc.dram_tensor("wq2", [N_EMBD, 64], BF16, kind="ExternalInput")
    WK2 = nc.dram_tensor("wk2", [N_EMBD, 64], BF16, kind="ExternalInput")
    WP1 = nc.dram_tensor("wp1", [128, N_EMBD], BF16, kind="ExternalInput")
    WP2 = nc.dram_tensor("wp2", [64, N_EMBD], BF16, kind="ExternalInput")
    Y = nc.dram_tensor("y", [T, N_EMBD], F32, kind="ExternalOutput")

    xt_ap = XT.ap().rearrange("(c p) t -> p c t", p=128)

    with tile.TileContext(nc) as tc:
        with (
            tc.tile_pool(name="const", bufs=1) as const_pool,
            tc.tile_pool(name="wpool", bufs=1) as wpool,
            tc.tile_pool(name="qkvt", bufs=1) as qkvt,
            tc.tile_pool(name="vsb", bufs=1) as vsb_pool,
            tc.tile_pool(name="ynt", bufs=1) as ynt_pool,
            tc.tile_pool(name="xs", bufs=3) as xs_pool,
            tc.tile_pool(name="ptp", bufs=4) as pt_pool,
            tc.tile_pool(name="rp", bufs=4) as r_pool,
            tc.tile_pool(name="yout", bufs=4) as yout_pool,
            tc.tile_pool(name="pbig", bufs=2, space="PSUM") as pbig,
            tc.tile_pool(name="pf", bufs=2, space="PSUM") as pf_pool,
            tc.tile_pool(name="py", bufs=2, space="PSUM") as py_pool,
        ):
            # ---- constants ----
            # causal triangle for diagonal 128x128 blocks: T[p, c] = 1 iff
            # c >= p (query col c sees key row p).
            tri_f = const_pool.tile([128, 128], F32, name="tri_f")
            nc.gpsimd.memset(tri_f, 1.0)
            nc.gpsimd.affine_select(
                out=tri_f, in_=tri_f,
                compare_op=mybir.AluOpType.is_ge,
                fill=0.0, base=0, channel_multiplier=-1,
                pattern=[[1, 128]],
            )
            tri = const_pool.tile([128, 128], BF16, name="tri")
            nc.vector.tensor_copy(out=tri, in_=tri_f)
            ones_rf = const_pool.tile([1, 64], F32, name="ones_rf")
            nc.vector.memset(ones_rf, 1.0)
            ones_r = const_pool.tile([1, 64], F32R, name="ones_r")
            nc.vector.tensor_copy(out=ones_r, in_=ones_rf)

            # ---- weights (bf16, direct load) ----
            def load_w(src_ap, shape, tag):
                t = wpool.tile(shape, BF16, tag=tag)
                nc.sync.dma_start(out=t, in_=src_ap)
                return t

            wq01r = load_w(WQ01.ap().rearrange("(c p) m -> p c m", p=128), [128, CCHUNKS, 128], "wq01r")
            wk01r = load_w(WK01.ap().rearrange("(c p) m -> p c m", p=128), [128, CCHUNKS, 128], "wk01r")
            wvr = load_w(WV.ap().rearrange("(c p) m -> p c m", p=128), [128, CCHUNKS, 192], "wvr")
            wq2r = load_w(WQ2.ap().rearrange("(c p) m -> p c m", p=128), [128, CCHUNKS, 64], "wq2r")
            wk2r = load_w(WK2.ap().rearrange("(c p) m -> p c m", p=128), [128, CCHUNKS, 64], "wk2r")
            wp1r = load_w(WP1.ap(), [128, N_EMBD], "wp1r")
            wp2r = load_w(WP2.ap(), [64, N_EMBD], "wp2r")

            # ---- persistent activations (bf16) ----
            QT01 = qkvt.tile([128, T], BF16, tag="qt01")
            KT01 = qkvt.tile([128, T], BF16, tag="kt01")
            Q2 = qkvt.tile([64, T], BF16, tag="q2")
            KT2 = qkvt.tile([64, T], BF16, tag="kt2")
            Vsb = vsb_pool.tile([128, n_kt, HEADS_PER_CORE, 65], BF16)
            YnT01 = ynt_pool.tile([128, T], BF16, tag="ynt01")
            YnT2 = ynt_pool.tile([64, T], BF16, tag="ynt2")

            ones_f = const_pool.tile([128, n_kt * HEADS_PER_CORE], F32)
            nc.vector.memset(ones_f, 1.0)
            nc.vector.tensor_copy(
                out=Vsb[:, :, :, 64:65].rearrange("p a b c -> p (a b c)"),
                in_=ones_f)

            scale = float(HEAD_DIM) ** -0.5

            # K jobs run early (every later query block needs them);
            # Q jobs are deferred to just before their query block.
            # V is computed directly in keys-major layout (v_unit below).
            kv_jobs = [(wk01r, KT01, 128), (wk2r, KT2, 64)]
            q_jobs = [(wq01r, QT01, 128), (wq2r, Q2, 64)]
            head_qk = [
                (QT01[0:64, :], KT01[0:64, :]),
                (QT01[64:128, :], KT01[64:128, :]),
                (Q2[0:64, :], KT2[0:64, :]),
            ]

            def mm(out, lhsT, rhs, start, stop):
                nc.tensor.ldweights(
                    lhsT,
                    tile_position=(lhsT.base_partition(), out.base_partition()))
                nc.tensor.matmul(out, lhsT, rhs, start=start, stop=stop)

            # ---- phase-B/B2/D work units (interleaved as PE filler) ----
            def b_unit(xs, ch, s, job, jobs):
                wt, out_sb, m = jobs[job]
                t0 = ch * TOK_CHUNK + s * SUB
                ps = pf_pool.tile([128, SUB], F32, tag="pf")
                for c in range(CCHUNKS):
                    mm(ps[0:m, :], wt[:, c, 0:m],
                       xs[:, c, s * SUB:(s + 1) * SUB],
                       start=(c == 0), stop=(c == CCHUNKS - 1))
                nc.vector.tensor_copy(out=out_sb[0:m, t0:t0 + SUB], in_=ps[0:m, :])

            def v_unit(xs, ch, kl):
                # V for one 128-key tile, all 3 heads, directly keys-major:
                # stationary = x tokens, moving = V weight columns.
                kt = ch * (TOK_CHUNK // KT) + kl
                pv = pf_pool.tile([128, 3, 64], F32, tag="pf")
                pv_flat = pv.rearrange("p h c -> p (h c)")
                for c in range(CCHUNKS):
                    mm(pv_flat, xs[:, c, kl * KT:(kl + 1) * KT],
                       wvr[:, c, :],
                       start=(c == 0), stop=(c == CCHUNKS - 1))
                nc.vector.tensor_copy(out=Vsb[:, kt, :, 0:64], in_=pv)

            def d_unit(tt):
                tsl = slice(tt * 128, (tt + 1) * 128)
                yo = yout_pool.tile([128, N_EMBD], F32)
                for c0, ncols in ((0, 512), (512, 256)):
                    pp = pf_pool.tile([128, 512], F32, tag="pf")
                    mm(pp[:, 0:ncols], YnT01[:, tsl],
                       wp1r[:, c0:c0 + ncols], start=True, stop=False)
                    mm(pp[:, 0:ncols], YnT2[0:64, tsl],
                       wp2r[0:64, c0:c0 + ncols], start=False, stop=True)
                    nc.vector.tensor_copy(out=yo[:, c0:c0 + ncols], in_=pp[:, 0:ncols])
                nc.sync.dma_start(out=Y.ap()[tsl, :], in_=yo)

            # ---- attention k-loop: software-pipelined emission ----
            def kloop(h, qs, yps):
                """One yield per batch of <=2 key tiles.  Each step emits
                S+exp of batch k, then mask+PV of batch k-1, so PE never
                sits behind ACT in program order.  Diagonal tiles are
                column-windowed and triangle-masked after exp."""
                qt_h, kt_h = head_qk[h]
                nkt = 4 * (qs + 1)
                qbase = qs * QSB
                batches = []
                full = list(range(nkt - 4))
                for i in range(0, len(full), 2):
                    batches.append([(kt, 0) for kt in full[i:i + 2]])
                batches.append([(nkt - 4, 0), (nkt - 3, 128)])
                batches.append([(nkt - 2, 256), (nkt - 1, 384)])

                def mask_pv(batch, pt):
                    off = 0
                    for (kt, d) in batch:
                        w = QSB - d
                        if kt >= nkt - 4:
                            nc.vector.tensor_mul(
                                pt[:, off:off + 128], pt[:, off:off + 128], tri)
                        off += w
                    off = 0
                    for (kt, d) in batch:
                        w = QSB - d
                        mm(yps[:, d:QSB], Vsb[:, kt, h, :], pt[:, off:off + w],
                           start=(kt == 0), stop=(kt == nkt - 1))
                        off += w

                prev = None
                for batch in batches:
                    widths = [QSB - d for (_, d) in batch]
                    W = sum(widths)
                    sb = pbig.tile([128, 1024], F32, tag="big")
                    off = 0
                    for (kt, d), w in zip(batch, widths):
                        mm(sb[:, off:off + w],
                           kt_h[:, kt * KT:(kt + 1) * KT],
                           qt_h[:, qbase + d:qbase + QSB],
                           start=True, stop=True)
                        off += w
                    pt = pt_pool.tile([128, 1024], BF16)
                    nc.scalar.activation(
                        out=pt[:, 0:W], in_=sb[:, 0:W],
                        func=mybir.ActivationFunctionType.Exp, scale=scale)
                    if prev is not None:
                        mask_pv(*prev)
                    prev = (batch, pt)
                    yield
                mask_pv(*prev)

            def finish(h, qs, yps):
                """Normalize in keys-major layout and store Y^T (bf16)."""
                qsl = slice(qs * QSB, (qs + 1) * QSB)
                rrow = r_pool.tile([1, QSB], F32R)
                with nc.allow_low_precision(reason="fp32r storage is fp32 bits"):
                    nc.vector.reciprocal(rrow, yps[64:65, :])
                rbp = pf_pool.tile([64, QSB], F32, tag="pf", name="rbp")
                nc.tensor.matmul(rbp, ones_r, rrow, start=True, stop=True)
                rb = r_pool.tile([64, QSB], F32, tag="rb", name="rb")
                nc.vector.tensor_copy(out=rb, in_=rbp)
                if h == 0:
                    dst = YnT01[0:64, qsl]
                elif h == 1:
                    dst = YnT01[64:128, qsl]
                else:
                    dst = YnT2[0:64, qsl]
                nc.vector.tensor_mul(dst, yps[0:64, :], rb)

            n_chunks = T // TOK_CHUNK

            for _ in range(repeat):
                from collections import deque
                bq, b2q, qq, dq = deque(), deque(), deque(), deque()
                chunks_queued = set()

                def queue_chunk(ch):
                    if ch in chunks_queued or ch >= n_chunks or 'B' not in phases:
                        return
                    chunks_queued.add(ch)
                    xs = xs_pool.tile([128, CCHUNKS, TOK_CHUNK], BF16)
                    nc.sync.dma_start(
                        out=xs,
                        in_=xt_ap[:, :, ch * TOK_CHUNK:(ch + 1) * TOK_CHUNK])
                    for s in range(TOK_CHUNK // SUB):
                        for job in range(len(kv_jobs)):
                            bq.append((ch, lambda xs=xs, ch=ch, s=s, job=job:
                                       b_unit(xs, ch, s, job, kv_jobs)))
                        sub_qs = ch * (TOK_CHUNK // SUB) + s
                        for job in range(len(q_jobs)):
                            qq.append((sub_qs, lambda xs=xs, ch=ch, s=s, job=job:
                                       b_unit(xs, ch, s, job, q_jobs)))
                    if 'B2' in phases:
                        for kl in range(TOK_CHUNK // KT):
                            kt = ch * (TOK_CHUNK // KT) + kl
                            b2q.append((kt, lambda xs=xs, ch=ch, kl=kl:
                                        v_unit(xs, ch, kl)))

                def force_prereqs(qs):
                    """Emit every queued B unit of chunks <= qs//2 and every
                    B2 unit with kt < 4*(qs+1)."""
                    need_ch = min(qs // 2, n_chunks - 1)
                    while bq and bq[0][0] <= need_ch:
                        bq.popleft()[1]()
                    while b2q and b2q[0][0] < 4 * (qs + 1):
                        b2q.popleft()[1]()
                    while qq and qq[0][0] <= qs:
                        qq.popleft()[1]()

                state = {'cur_qs': 0}

                def drain_fillers(n, final=False):
                    for _ in range(n):
                        if bq:
                            bq.popleft()[1]()
                        elif b2q:
                            b2q.popleft()[1]()
                        elif dq:
                            dq.popleft()()
                        elif final and qq:
                            qq.popleft()[1]()
                        else:
                            return

                def queue_d(qs):
                    if 'D' not in phases:
                        return
                    for tt in range(4 * qs, 4 * qs + 4):
                        dq.append(lambda tt=tt: d_unit(tt))

                # prologue: first token chunk (K, V, Q for qs 0-1)
                queue_chunk(0)
                force_prereqs(1)
                queue_chunk(1)

                if 'C' in phases:
                    loop_list = [(h, qs) for qs in range(n_qsb)
                                 for h in range(HEADS_PER_CORE)]
                else:
                    loop_list = []
                    while bq or b2q or qq or dq:
                        drain_fillers(8, final=True)
                active = []
                next_i = 0
                qs_done = {}
                while active or next_i < len(loop_list):
                    while len(active) < 2 and next_i < len(loop_list):
                        h, qs = loop_list[next_i]
                        state['cur_qs'] = qs
                        force_prereqs(qs)
                        queue_chunk(qs // 2 + 1)
                        yps = py_pool.tile([65, QSB], F32, tag="y",
                                           name=f"yps{h}_{qs}")
                        active.append([kloop(h, qs, yps), h, qs, yps])
                        next_i += 1
                    for entry in list(active):
                        gen, h, qs, yps = entry
                        try:
                            next(gen)
                        except StopIteration:
                            finish(h, qs, yps)
                            qs_done[qs] = qs_done.get(qs, 0) + 1
                            if qs_done[qs] == HEADS_PER_CORE:
                                queue_d(qs)
                            active.remove(entry)
                    drain_fillers(2)
                # epilogue: leftover projection units
                while bq or b2q or qq or dq:
                    drain_fillers(8, final=True)

    nc.compile()
    return nc


def make_in_maps(x, w_qkv, w_proj, T=T_FULL):
    """Per-core input dicts from full inputs (numpy), converted to bf16."""
    import ml_dtypes
    bf16 = ml_dtypes.bfloat16
    x = np.asarray(x, dtype=np.float32)
    w_qkv = np.asarray(w_qkv, dtype=np.float32).astype(bf16)
    w_proj = np.asarray(w_proj, dtype=np.float32).astype(bf16)
    cores_per_batch = N_CORES // B
    xt_b = [np.ascontiguousarray(x[b].T.astype(bf16)) for b in range(B)]
    in_maps = []
    for core in range(N_CORES):
        b = core // cores_per_batch
        h0 = (core % cores_per_batch) * HEADS_PER_CORE
        h1, h2 = h0 + 1, h0 + 2
        col = lambda kind, h: w_qkv[:, kind * N_EMBD + h * HEAD_DIM:
                                    kind * N_EMBD + (h + 1) * HEAD_DIM]
        in_maps.append({
            "xt": xt_b[b],
            "wq01": np.ascontiguousarray(np.concatenate([col(0, h0), col(0, h1)], axis=1)),
            "wk01": np.ascontiguousarray(np.concatenate([col(1, h0), col(1, h1)], axis=1)),
            "wv": np.ascontiguousarray(np.concatenate([col(2, h0), col(2, h1), col(2, h2)], axis=1)),
            "wq2": np.ascontiguousarray(col(0, h2)),
            "wk2": np.ascontiguousarray(col(1, h2)),
            "wp1": np.ascontiguousarray(w_proj[h0 * HEAD_DIM:(h1 + 1) * HEAD_DIM, :]),
            "wp2": np.ascontiguousarray(w_proj[h2 * HEAD_DIM:(h2 + 1) * HEAD_DIM, :]),
        })
    return in_maps


def gather_output(results, T=T_FULL):
    cores_per_batch = N_CORES // B
    out = np.empty((B, T, N_EMBD), dtype=np.float32)
    for b in range(B):
        parts = [results[b * cores_per_batch + j]["y"] for j in range(cores_per_batch)]
        out[b] = parts[0] + parts[1] + parts[2] + parts[3]
    return out


_CACHE = {}


def _get_nc(T=T_FULL, repeat=1):
    key = (T, repeat)
    if key not in _CACHE:
        _CACHE[key] = build_nc(T, repeat)
    return _CACHE[key]


def kernel(x, w_qkv, w_proj):
    import time as _time
    from concourse.bass_utils import run_bass_kernel_spmd
    T = x.shape[1]
    nc = _get_nc(T)
    in_maps = make_in_maps(x, w_qkv, w_proj, T)
    last_err = None
    for attempt in range(3):
        try:
            res = run_bass_kernel_spmd(nc, in_maps, list(range(N_CORES)))
            return gather_output(res.results, T)
        except Exception as e:  # transient device wedge: retry after a pause
            last_err = e
            _time.sleep(20 * (attempt + 1))
    raise last_err
